# revision 12
# baseline (speedup 1.0000x reference)
"""Trainium2 Bass kernel: DiscreteEmbedding (rect-window embedding lookup).

Math (matches the jax reference semantics):
    y = x*2048 + 0.5
    i_lo = ceil(y)-1, boundary b = (y integer)
    out[t] = TC[i_lo + 2049*b]  where TC (host-prebuilt, fp16) is
      TC[0:2048] = T, TC[2048] = 0, TC[2049+k] = (T[k]+T[k+1])/2 (T[2048]=0)

Device strategy (8 cores, data-parallel over tokens):
  - TC is prebuilt on host (weight prep) and passed as an fp16 input;
    the device does NO table work: load x, index math, gather, store.
  - fp16 rows (256B) halve gather/store bytes vs fp32, taking the 16
    DMA engines out of saturation (measured ~29.5ns/512B packet).
  - Measured limits that set the floor (~46us): the Q7 SWDGE library
    load (~10us, hidden behind x-load + idx math; gathers can't start
    before ~17us) and the per-SWDGE-queue descriptor-pair dispatch
    pacing (~10.8ns/idx/queue, byte-size independent; 8192 idx over
    the 4-queue ucode max -> ~22us drain). single_packet, transpose
    mode, and fp16 vs fp32 all leave that pacing unchanged.
  - Index math on DVE in 2 column slices; gather chunks round-robin
    the 4 queues in small chunks so desc-gen never blocks the Pool
    engine long enough to starve a queue.
  - Stores alternate between the SP and ACT HWDGE rings, fp16.
  - Host un-permutes rows and upcasts fp16->fp32 while un-sharding.
"""

import numpy as np

import concourse.mybir as mybir
import concourse.tile as tile
from concourse import bacc, bass_utils

N_CORES = 8
B, S = 32, 2048
V, D = 2048, 128
TOK = B * S                 # 65536 tokens total
TPC = TOK // N_CORES        # 8192 tokens per core
SPC = TPC // 16             # 512: free dim of the wrapped [16, 512] x layout
ABASE = V + 1               # 2049: base row of the averaged-pair table
VEXT = 2 * V + 1            # 4097 TC rows
NQ = 4                      # SWDGE queues
NSLICE = 2                  # idx-math column slices
JB = TPC // 128             # 64 j-blocks of 128 tokens
# per idx slice: j-blocks round-robin over the 4 queues in small chunks so
# every ring stays fed; totals skewed (+1jb q0 / -1jb q3 overall) so the
# staggered per-queue drains finish together
CHUNKS_A = [(4, 0), (4, 1), (4, 2), (4, 3), (5, 0), (4, 1), (4, 2), (3, 3)]
CHUNKS_B = [(4, 0), (4, 1), (4, 2), (4, 3), (4, 0), (4, 1), (4, 2), (4, 3)]

F32 = mybir.dt.float32
F16 = mybir.dt.float16
I32 = mybir.dt.int32
I16 = mybir.dt.int16
OP = mybir.AluOpType


def build():
    nc = bacc.Bacc(
        "TRN2",
        target_bir_lowering=False,
        debug=False,
        num_devices=N_CORES,
        num_swdge_queues=NQ,
    )
    xr = nc.dram_tensor("xr", [128, SPC], F32, kind="ExternalInput")
    tc_tbl = nc.dram_tensor("tc", [VEXT, D], F16, kind="ExternalInput")
    out = nc.dram_tensor("out", [TPC, D], F16, kind="ExternalOutput")

    with tile.TileContext(nc) as tc:
        with tc.tile_pool(name="sb", bufs=1) as sb, tc.tile_pool(name="g", bufs=1) as gp:
            zidx = sb.tile([128, 16], I16)
            nc.gpsimd.memset(zidx[:], 0)
            # warm-up: pay the Q7 library-load + per-queue ring init early.
            for q in range(NQ):
                wg = sb.tile([128, D], F16, tag=f"warm{q}")
                nc.gpsimd.dma_gather(
                    wg[:].rearrange("p (j d) -> p j d", d=D),
                    tc_tbl[:],
                    zidx[:, 0:1],
                    num_idxs=16,
                    num_idxs_reg=16,
                    elem_size=D,
                    single_packet=False,
                    queue_num=q,
                )

            xt = sb.tile([128, SPC], F32)
            nc.sync.dma_start(out=xt[:], in_=xr[:])

            out_v = out[:].rearrange("(p j) d -> p (j d)", p=128)
            nidx_regs = {n: nc.gpsimd.to_reg(128 * n)
                         for n in sorted({c[0] for c in CHUNKS_A + CHUNKS_B})}
            store_rings = [nc.sync, nc.scalar]
            cols = SPC // NSLICE        # 256 columns per slice
            for s in range(NSLICE):
                c0, c1 = s * cols, (s + 1) * cols
                # ---- index math (fp32, exact): y = x*2048 + 0.5 ----
                y = sb.tile([128, cols], F32, tag=f"y{s}")
                nc.vector.tensor_scalar(y[:], xt[:, c0:c1], 2048.0, 0.5, op0=OP.mult, op1=OP.add)
                # i0 = int(y) rounded to SOME neighboring integer; fix up with
                # exact fp32 compares (robust to the HW float->int round mode).
                i0 = sb.tile([128, cols], I32, tag=f"i0{s}")
                nc.vector.tensor_copy(i0[:], y[:])
                f0 = sb.tile([128, cols], F32, tag=f"f0{s}")
                nc.vector.tensor_copy(f0[:], i0[:])
                lt = sb.tile([128, cols], F32, tag=f"lt{s}")    # f0 < y
                nc.vector.tensor_tensor(lt[:], f0[:], y[:], op=OP.is_lt)
                bnd = sb.tile([128, cols], F32, tag=f"bnd{s}")  # y integer -> blend row
                nc.vector.tensor_tensor(bnd[:], f0[:], y[:], op=OP.is_equal)
                # i_lo = ceil(y) - 1 = f0 + lt - 1
                lf = sb.tile([128, cols], F32, tag=f"lf{s}")
                nc.vector.scalar_tensor_tensor(
                    out=lf[:], in0=lt[:], scalar=-1.0, in1=f0[:], op0=OP.add, op1=OP.add
                )
                # idx2 = i_lo + 2049*b
                idxf = sb.tile([128, cols], F32, tag=f"idxf{s}")
                nc.vector.scalar_tensor_tensor(
                    out=idxf[:], in0=bnd[:], scalar=float(ABASE), in1=lf[:],
                    op0=OP.mult, op1=OP.add,
                )
                idx16 = sb.tile([128, cols], I16, tag=f"idx16{s}")
                nc.vector.tensor_copy(idx16[:], idxf[:])

                # ---- chunked gather + store for this slice ----
                j0 = s * (JB // NSLICE)
                lc = 0
                for ci, (jbc, q) in enumerate(CHUNKS_A if s == 0 else CHUNKS_B):
                    g = gp.tile([128, jbc * D], F16, tag=f"g{s}_{ci}")
                    nc.gpsimd.dma_gather(
                        g[:].rearrange("p (j d) -> p j d", d=D),
                        tc_tbl[:],
                        idx16[:, lc * 8 : (lc + jbc) * 8],
                        num_idxs=128 * jbc,
                        num_idxs_reg=nidx_regs[jbc],
                        elem_size=D,
                        single_packet=False,
                        queue_num=q,
                    )
                    ring = store_rings[ci % len(store_rings)]
                    ring.dma_start(
                        out=out_v[:, (j0 + lc) * D : (j0 + lc + jbc) * D], in_=g[:]
                    )
                    lc += jbc
                assert lc == JB // NSLICE
    nc.compile()
    return nc


_NC = None


def _row_perm():
    """out row r holds gather position i(r); position i handles token
    t(i) = (i%16)*512 + i//16 (x wrapped [16,512] across partitions)."""
    r = np.arange(TPC)
    p, j = r // JB, r % JB
    i = j * 128 + p
    return (i % 16) * SPC + i // 16  # token index held at row r


def _build_tc(t):
    """Host weight prep: combined fp16 table (plain rows + zero + pair avgs)."""
    tz = np.vstack([t, np.zeros((1, D), np.float32)])
    avg = 0.5 * (tz[:-1] + tz[1:])
    return np.ascontiguousarray(np.vstack([tz, avg]).astype(np.float16))


def kernel(x, time_embedding):
    global _NC
    x = np.ascontiguousarray(np.asarray(x, dtype=np.float32))
    t = np.ascontiguousarray(np.asarray(time_embedding, dtype=np.float32))
    tc16 = _build_tc(t)
    xf = x.reshape(-1)
    in_maps = []
    for c in range(N_CORES):
        xc = xf[c * TPC : (c + 1) * TPC].reshape(16, SPC)
        in_maps.append({"xr": np.ascontiguousarray(np.tile(xc, (8, 1))), "tc": tc16})

    if _NC is None:
        _NC = build()
    res = bass_utils.run_bass_kernel_spmd(_NC, in_maps, core_ids=list(range(N_CORES)))
    global _LAST_RES
    _LAST_RES = res

    tkn = _row_perm()
    outs = []
    for c in range(N_CORES):
        oc = np.asarray(res.results[c]["out"])
        full = np.empty_like(oc)
        full[tkn] = oc
        outs.append(full)
    return np.concatenate(outs, axis=0).astype(np.float32).reshape(B, S, D)


# revision 13
# speedup vs baseline: 1.0221x; 1.0221x over previous
"""Trainium2 Bass kernel: DiscreteEmbedding (rect-window embedding lookup).

Math (matches the jax reference semantics):
    y = x*2048 + 0.5
    i_lo = ceil(y)-1, boundary b = (y integer)
    out[t] = TC[i_lo + 2049*b]  where TC (host-prebuilt, fp16) is
      TC[0:2048] = T, TC[2048] = 0, TC[2049+k] = (T[k]+T[k+1])/2 (T[2048]=0)

Device strategy (8 cores, data-parallel over tokens):
  - TC is prebuilt on host (weight prep) and passed as an fp16 input;
    the device does NO table work: load x, index math, gather, store.
  - fp16 rows (256B) halve gather/store bytes vs fp32, taking the 16
    DMA engines out of saturation (measured ~29.5ns/512B packet).
  - Measured limits that set the floor (~46us): the Q7 SWDGE library
    load (~10us, hidden behind x-load + idx math; gathers can't start
    before ~17us) and the per-SWDGE-queue descriptor-pair dispatch
    pacing (~10.8ns/idx/queue, byte-size independent; 8192 idx over
    the 4-queue ucode max -> ~22us drain). single_packet, transpose
    mode, and fp16 vs fp32 all leave that pacing unchanged.
  - Index math on DVE in 2 column slices; gather chunks round-robin
    the 4 queues in small chunks so desc-gen never blocks the Pool
    engine long enough to starve a queue.
  - Stores alternate between the SP and ACT HWDGE rings, fp16.
  - Host un-permutes rows and upcasts fp16->fp32 while un-sharding.
"""

import numpy as np

import concourse.mybir as mybir
import concourse.tile as tile
from concourse import bacc, bass_utils

N_CORES = 8
B, S = 32, 2048
V, D = 2048, 128
TOK = B * S                 # 65536 tokens total
TPC = TOK // N_CORES        # 8192 tokens per core
SPC = TPC // 16             # 512: free dim of the wrapped [16, 512] x layout
ABASE = V + 1               # 2049: base row of the averaged-pair table
VEXT = 2 * V + 1            # 4097 TC rows
NQ = 4                      # SWDGE queues
NSLICE = 2                  # idx-math column slices
JB = TPC // 128             # 64 j-blocks of 128 tokens
# The drain is paced by Q7 descriptor GENERATION (~9.5ns/idx per queue
# core-pair, concurrent across queues); a chunk's packets only fire at its
# gen completion. So: equal per-queue totals (gen is queue-independent),
# chunks small enough to fit the desc ring (<=640 idx — bigger serializes
# queue dispatch), and a TINY final chunk per queue so the last
# gen+burst+sem+store tail is short.
CHUNKS_A = [(4, 0), (4, 1), (4, 2), (4, 3), (4, 0), (4, 1), (4, 2), (4, 3)]
CHUNKS_B = [(4, 0), (4, 1), (4, 2), (4, 3), (3, 0), (3, 1), (3, 2), (3, 3),
            (1, 0), (1, 1), (1, 2), (1, 3)]

F32 = mybir.dt.float32
F16 = mybir.dt.float16
I32 = mybir.dt.int32
I16 = mybir.dt.int16
OP = mybir.AluOpType


def build():
    nc = bacc.Bacc(
        "TRN2",
        target_bir_lowering=False,
        debug=False,
        num_devices=N_CORES,
        num_swdge_queues=NQ,
    )
    xr = nc.dram_tensor("xr", [128, SPC], F32, kind="ExternalInput")
    tc_tbl = nc.dram_tensor("tc", [VEXT, D], F16, kind="ExternalInput")
    out = nc.dram_tensor("out", [TPC, D], F16, kind="ExternalOutput")

    with tile.TileContext(nc) as tc:
        with tc.tile_pool(name="sb", bufs=1) as sb, tc.tile_pool(name="g", bufs=1) as gp:
            zidx = sb.tile([128, 16], I16)
            nc.gpsimd.memset(zidx[:], 0)
            # warm-up: pay the Q7 library-load + per-queue ring init early.
            for q in range(NQ):
                wg = sb.tile([128, D], F16, tag=f"warm{q}")
                nc.gpsimd.dma_gather(
                    wg[:].rearrange("p (j d) -> p j d", d=D),
                    tc_tbl[:],
                    zidx[:, 0:1],
                    num_idxs=16,
                    num_idxs_reg=16,
                    elem_size=D,
                    single_packet=False,
                    queue_num=q,
                )

            xt = sb.tile([128, SPC], F32)
            nc.sync.dma_start(out=xt[:], in_=xr[:])

            out_v = out[:].rearrange("(p j) d -> p (j d)", p=128)
            nidx_regs = {n: nc.gpsimd.to_reg(128 * n)
                         for n in sorted({c[0] for c in CHUNKS_A + CHUNKS_B})}
            store_rings = [nc.sync, nc.scalar]
            cols = SPC // NSLICE        # 256 columns per slice
            for s in range(NSLICE):
                c0, c1 = s * cols, (s + 1) * cols
                # ---- index math (fp32, exact): y = x*2048 + 0.5 ----
                y = sb.tile([128, cols], F32, tag=f"y{s}")
                nc.vector.tensor_scalar(y[:], xt[:, c0:c1], 2048.0, 0.5, op0=OP.mult, op1=OP.add)
                # i0 = int(y) rounded to SOME neighboring integer; fix up with
                # exact fp32 compares (robust to the HW float->int round mode).
                i0 = sb.tile([128, cols], I32, tag=f"i0{s}")
                nc.vector.tensor_copy(i0[:], y[:])
                f0 = sb.tile([128, cols], F32, tag=f"f0{s}")
                nc.vector.tensor_copy(f0[:], i0[:])
                lt = sb.tile([128, cols], F32, tag=f"lt{s}")    # f0 < y
                nc.vector.tensor_tensor(lt[:], f0[:], y[:], op=OP.is_lt)
                bnd = sb.tile([128, cols], F32, tag=f"bnd{s}")  # y integer -> blend row
                nc.vector.tensor_tensor(bnd[:], f0[:], y[:], op=OP.is_equal)
                # i_lo = ceil(y) - 1 = f0 + lt - 1
                lf = sb.tile([128, cols], F32, tag=f"lf{s}")
                nc.vector.scalar_tensor_tensor(
                    out=lf[:], in0=lt[:], scalar=-1.0, in1=f0[:], op0=OP.add, op1=OP.add
                )
                # idx2 = i_lo + 2049*b
                idxf = sb.tile([128, cols], F32, tag=f"idxf{s}")
                nc.vector.scalar_tensor_tensor(
                    out=idxf[:], in0=bnd[:], scalar=float(ABASE), in1=lf[:],
                    op0=OP.mult, op1=OP.add,
                )
                idx16 = sb.tile([128, cols], I16, tag=f"idx16{s}")
                nc.vector.tensor_copy(idx16[:], idxf[:])

                # ---- chunked gather + store for this slice ----
                j0 = s * (JB // NSLICE)
                lc = 0
                for ci, (jbc, q) in enumerate(CHUNKS_A if s == 0 else CHUNKS_B):
                    g = gp.tile([128, jbc * D], F16, tag=f"g{s}_{ci}")
                    nc.gpsimd.dma_gather(
                        g[:].rearrange("p (j d) -> p j d", d=D),
                        tc_tbl[:],
                        idx16[:, lc * 8 : (lc + jbc) * 8],
                        num_idxs=128 * jbc,
                        num_idxs_reg=nidx_regs[jbc],
                        elem_size=D,
                        single_packet=False,
                        queue_num=q,
                    )
                    ring = store_rings[ci % len(store_rings)]
                    ring.dma_start(
                        out=out_v[:, (j0 + lc) * D : (j0 + lc + jbc) * D], in_=g[:]
                    )
                    lc += jbc
                assert lc == JB // NSLICE
    nc.compile()
    return nc


_NC = None


def _row_perm():
    """out row r holds gather position i(r); position i handles token
    t(i) = (i%16)*512 + i//16 (x wrapped [16,512] across partitions)."""
    r = np.arange(TPC)
    p, j = r // JB, r % JB
    i = j * 128 + p
    return (i % 16) * SPC + i // 16  # token index held at row r


def _build_tc(t):
    """Host weight prep: combined fp16 table (plain rows + zero + pair avgs)."""
    tz = np.vstack([t, np.zeros((1, D), np.float32)])
    avg = 0.5 * (tz[:-1] + tz[1:])
    return np.ascontiguousarray(np.vstack([tz, avg]).astype(np.float16))


def kernel(x, time_embedding):
    global _NC
    x = np.ascontiguousarray(np.asarray(x, dtype=np.float32))
    t = np.ascontiguousarray(np.asarray(time_embedding, dtype=np.float32))
    tc16 = _build_tc(t)
    xf = x.reshape(-1)
    in_maps = []
    for c in range(N_CORES):
        xc = xf[c * TPC : (c + 1) * TPC].reshape(16, SPC)
        in_maps.append({"xr": np.ascontiguousarray(np.tile(xc, (8, 1))), "tc": tc16})

    if _NC is None:
        _NC = build()
    res = bass_utils.run_bass_kernel_spmd(_NC, in_maps, core_ids=list(range(N_CORES)))
    global _LAST_RES
    _LAST_RES = res

    tkn = _row_perm()
    outs = []
    for c in range(N_CORES):
        oc = np.asarray(res.results[c]["out"])
        full = np.empty_like(oc)
        full[tkn] = oc
        outs.append(full)
    return np.concatenate(outs, axis=0).astype(np.float32).reshape(B, S, D)
